# revision 6
# baseline (speedup 1.0000x reference)
"""Multi-head attention (B=4, S=2048, D=1024, H=16) on 8 trn2 NeuronCores.

Sharding: core c = 2*b + g handles batch b, head-group g (8 heads, 512 dims).
Q/K/V projections are column-sharded (Megatron), Wo row-sharded; the Wo
partial sums for the two head-groups of each batch are reduced host-side.

Device layout notes:
  - All activations live in "transposed" [feature, seq] layout so every
    matmul has its contraction dim on SBUF partitions.  Host pre-transposes.
  - Scores are computed as S^T [k, q] tiles; softmax denominators come from
    an extra ones-column appended to V (PV matmul computes [O^T; rowsum]).
  - exp is issued on [128, 1024] PSUM tiles (2 k-chunks) to amortize ACT
    instruction overhead; no max-subtraction (scores are O(1) by
    construction; masked entries are zeroed multiplicatively after exp).
"""

import numpy as np

import concourse.mybir as mybir
import concourse.tile as tile
from concourse import bacc
from concourse.bass_utils import run_bass_kernel_spmd

F32 = mybir.dt.float32
B, S, D, H, DK = 4, 2048, 1024, 16, 64
DL = 512  # dims per head-group (8 heads * 64)
NH = 8  # heads per core
NHP = 4  # head pairs per core
NDI = D // 128  # 8   d_model chunks
NSC = S // 128  # 16  seq chunks (k side)
NSB = S // 512  # 4   seq blocks (q side)
NDC = DL // 128  # 4  local-dim chunks
EXP_GRP = 2  # k-chunks per exp instruction


def build_program(variant, reps=1):
    """variant: 'causal' | 'ones' | 'general'"""
    assert variant in ("causal", "ones", "general")
    nc = bacc.Bacc("TRN2", target_bir_lowering=False, debug=False)

    qT_d = nc.dram_tensor("qT", [D, S], F32, kind="ExternalInput")
    kT_d = nc.dram_tensor("kT", [D, S], F32, kind="ExternalInput")
    vT_d = nc.dram_tensor("vT", [D, S], F32, kind="ExternalInput")
    wq_d = nc.dram_tensor("wq", [D, DL], F32, kind="ExternalInput")
    wk_d = nc.dram_tensor("wk", [D, DL], F32, kind="ExternalInput")
    wv_d = nc.dram_tensor("wv", [D, DL], F32, kind="ExternalInput")
    wo_d = nc.dram_tensor("wo", [DL, D], F32, kind="ExternalInput")
    bq_d = nc.dram_tensor("bq", [128, NDC], F32, kind="ExternalInput")
    bk_d = nc.dram_tensor("bk", [128, NDC], F32, kind="ExternalInput")
    bv_d = nc.dram_tensor("bv", [128, DL], F32, kind="ExternalInput")
    if variant == "causal":
        mt_d = nc.dram_tensor("maskt", [4, 128, 512], F32, kind="ExternalInput")
    elif variant == "general":
        mT_d = nc.dram_tensor("maskT", [S, S], F32, kind="ExternalInput")
    outT_d = nc.dram_tensor("outT", [D, S], F32, kind="ExternalOutput")

    def kept_kcs(qb):
        return list(range(4 * qb + 4)) if variant == "causal" else list(range(NSC))

    with tile.TileContext(nc) as tc:
        for _rep in range(reps):
            with (
                tc.tile_pool(name="persist", bufs=1) as pers,
                tc.tile_pool(name="vt", bufs=1) as vtp,
                tc.tile_pool(name="stream", bufs=1) as stream,
                tc.tile_pool(name="qblk", bufs=2) as qbp,
                tc.tile_pool(name="xblk", bufs=2) as xbp,
                tc.tile_pool(name="epool", bufs=3) as ep,
                tc.tile_pool(name="rpool", bufs=2) as rp,
                tc.tile_pool(name="ostage", bufs=3) as osp,
            ):
                # ---- constants ----
                bq_sb = pers.tile([128, NDC], F32, tag="bq")
                bk_sb = pers.tile([128, NDC], F32, tag="bk")
                bv_sb = pers.tile([128, DL], F32, tag="bv")
                nc.sync.dma_start(out=bq_sb[:], in_=bq_d.ap())
                nc.sync.dma_start(out=bk_sb[:], in_=bk_d.ap())
                nc.sync.dma_start(out=bv_sb[:], in_=bv_d.ap())
                if variant == "causal":
                    mt_sb = pers.tile([128, 4, 512], F32, tag="mt")
                    nc.sync.dma_start(
                        out=mt_sb[:], in_=mt_d.ap().rearrange("j p q -> p j q")
                    )
                wo_sb = pers.tile([128, NDC, D], F32, tag="wo")
                nc.sync.dma_start(
                    out=wo_sb[:], in_=wo_d.ap().rearrange("(c p) e -> p c e", p=128)
                )

                Ksb = pers.tile([128, NDC, S], F32, tag="Ksb")
                vts = [
                    vtp.tile([128, NH * 65], F32, tag=f"vt{sc}", name=f"vt{sc}")
                    for sc in range(NSC)
                ]

                # ---- K projection:  Ksb[p, dc, s] = (key @ Wk_g.T).T + bk ----
                with (
                    tc.tile_pool(name="wkp", bufs=1) as wkp,
                    tc.tile_pool(name="ppk", bufs=4, space="PSUM") as ppk,
                ):
                    wk_sb = wkp.tile([128, NDI, DL], F32, tag="wk")
                    nc.sync.dma_start(
                        out=wk_sb[:], in_=wk_d.ap().rearrange("(c p) d -> p c d", p=128)
                    )
                    for sb in range(NSB):
                        kt = stream.tile([128, NDI, 512], F32, tag="qkstream")
                        nc.sync.dma_start(
                            out=kt[:],
                            in_=kT_d.ap()[:, sb * 512 : (sb + 1) * 512].rearrange(
                                "(c p) s -> p c s", p=128
                            ),
                        )
                        for dc in range(NDC):
                            ps = ppk.tile([128, 512], F32, tag="ppk")
                            for di in range(NDI):
                                nc.tensor.matmul(
                                    ps[:],
                                    wk_sb[:, di, dc * 128 : (dc + 1) * 128],
                                    kt[:, di, :],
                                    start=(di == 0),
                                    stop=(di == NDI - 1),
                                )
                            nc.vector.tensor_scalar_add(
                                Ksb[:, dc, sb * 512 : (sb + 1) * 512],
                                ps[:],
                                bk_sb[:, dc : dc + 1],
                            )

                # ---- V projection into [V | 1] tiles ----
                with (
                    tc.tile_pool(name="wvp", bufs=1) as wvp,
                    tc.tile_pool(name="vstream", bufs=2) as vstream,
                    tc.tile_pool(name="ppv", bufs=4, space="PSUM") as ppv,
                ):
                    wv_sb = wvp.tile([128, NDI, DL], F32, tag="wv")
                    nc.sync.dma_start(
                        out=wv_sb[:], in_=wv_d.ap().rearrange("(c p) d -> p c d", p=128)
                    )
                    for sc in range(NSC):
                        vtr = vstream.tile([128, NDI, 128], F32, tag="vtr")
                        nc.sync.dma_start(
                            out=vtr[:],
                            in_=vT_d.ap()[:, sc * 128 : (sc + 1) * 128].rearrange(
                                "(c p) s -> p c s", p=128
                            ),
                        )
                        ps = ppv.tile([128, DL], F32, tag="ppv")
                        for di in range(NDI):
                            nc.tensor.matmul(
                                ps[:],
                                vtr[:, di, :],
                                wv_sb[:, di, :],
                                start=(di == 0),
                                stop=(di == NDI - 1),
                            )
                        vt3 = vts[sc][:].rearrange("p (h c) -> p h c", h=NH)
                        nc.vector.tensor_add(
                            vt3[:, :, 0:64],
                            ps[:].rearrange("p (h c) -> p h c", h=NH),
                            bv_sb[:].rearrange("p (h c) -> p h c", h=NH),
                        )
                        nc.any.memset(vt3[:, :, 64:65], 1.0)

                # ---- per q-block: Q proj, attention, out proj ----
                with (
                    tc.tile_pool(name="wqp", bufs=1) as wqp,
                    tc.tile_pool(name="mq", bufs=1) as mqp,
                    tc.tile_pool(name="ppq", bufs=1, space="PSUM") as ppq,
                    tc.tile_pool(name="pss", bufs=1, space="PSUM") as pss,
                    tc.tile_pool(name="pso", bufs=1, space="PSUM") as pso,
                    tc.tile_pool(name="ppc", bufs=1, space="PSUM") as ppc,
                ):
                    wq_sb = wqp.tile([128, NDI, DL], F32, tag="wq")
                    nc.sync.dma_start(
                        out=wq_sb[:], in_=wq_d.ap().rearrange("(c p) d -> p c d", p=128)
                    )
                    for qb in range(NSB):
                        qsl = slice(qb * 512, (qb + 1) * 512)
                        # Q projection for this q block
                        qt = stream.tile([128, NDI, 512], F32, tag="qkstream")
                        nc.sync.dma_start(
                            out=qt[:],
                            in_=qT_d.ap()[:, qsl].rearrange("(c p) s -> p c s", p=128),
                        )
                        Qblk = qbp.tile([128, NDC, 512], F32, tag="Qblk")
                        for dc in range(NDC):
                            ps = ppq.tile([128, 512], F32, tag="ppq")
                            for di in range(NDI):
                                nc.tensor.matmul(
                                    ps[:],
                                    wq_sb[:, di, dc * 128 : (dc + 1) * 128],
                                    qt[:, di, :],
                                    start=(di == 0),
                                    stop=(di == NDI - 1),
                                )
                            nc.vector.tensor_scalar_add(
                                Qblk[:, dc, :], ps[:], bq_sb[:, dc : dc + 1]
                            )

                        if variant == "general":
                            mq_sb = mqp.tile([128, NSC, 512], F32, tag="mq")
                            nc.sync.dma_start(
                                out=mq_sb[:],
                                in_=mT_d.ap()[:, qsl].rearrange(
                                    "(c p) q -> p c q", p=128
                                ),
                            )

                        kept = kept_kcs(qb)
                        groups = [
                            kept[i : i + EXP_GRP] for i in range(0, len(kept), EXP_GRP)
                        ]
                        first, last = kept[0], kept[-1]
                        Xblk = xbp.tile([128, NDC, 512], F32, tag="Xblk")
                        for hp in range(NHP):
                            ps_o = {}
                            ps_o[0] = pso.tile([65, 512], F32, tag="oA", name="psoA")
                            ps_o[1] = pso.tile([65, 512], F32, tag="oB", name="psoB")
                            for grp in groups:
                                ng = len(grp)
                                ps_s = {
                                    0: pss.tile([128, EXP_GRP * 512], F32, tag="sA", name="pssA"),
                                    1: pss.tile([128, EXP_GRP * 512], F32, tag="sB", name="pssB"),
                                }
                                for hb, (p0, p1) in enumerate(((0, 64), (64, 128))):
                                    for j, kc in enumerate(grp):
                                        nc.tensor.matmul(
                                            ps_s[hb][:, j * 512 : (j + 1) * 512],
                                            Ksb[p0:p1, hp, kc * 128 : (kc + 1) * 128],
                                            Qblk[p0:p1, hp, :],
                                            start=True,
                                            stop=True,
                                        )
                                es = {}
                                for hb in range(2):
                                    et = ep.tile(
                                        [128, EXP_GRP * 512], F32, tag="e",
                                        name=f"e{hb}"
                                    )
                                    nc.scalar.activation(
                                        et[:, : ng * 512],
                                        ps_s[hb][:, : ng * 512],
                                        mybir.ActivationFunctionType.Exp,
                                        scale=1.0 / np.sqrt(DK),
                                    )
                                    es[hb] = et
                                for j, kc in enumerate(grp):
                                    esl = slice(j * 512, (j + 1) * 512)
                                    if variant == "general":
                                        for hb in range(2):
                                            nc.vector.tensor_mul(
                                                es[hb][:, esl],
                                                es[hb][:, esl],
                                                mq_sb[:, kc, :],
                                            )
                                    elif variant == "causal" and kc >= 4 * qb:
                                        for hb in range(2):
                                            nc.vector.tensor_mul(
                                                es[hb][:, esl],
                                                es[hb][:, esl],
                                                mt_sb[:, kc - 4 * qb, :],
                                            )
                                for hb in range(2):
                                    for j, kc in enumerate(grp):
                                        h = 2 * hp + hb
                                        nc.tensor.matmul(
                                            ps_o[hb][:],
                                            vts[kc][:, h * 65 : (h + 1) * 65],
                                            es[hb][:, j * 512 : (j + 1) * 512],
                                            start=(kc == first),
                                            stop=(kc == last),
                                        )
                            for hb, (p0, p1) in enumerate(((0, 64), (64, 128))):
                                r = rp.tile([1, 512], F32, tag="r", name=f"r{hb}")
                                rb = rp.tile([64, 512], F32, tag="rb", name=f"rb{hb}")
                                nc.vector.reciprocal(r[:], ps_o[hb][64:65, :])
                                nc.gpsimd.partition_broadcast(rb[:], r[0:1, :])
                                nc.vector.tensor_mul(
                                    Xblk[p0:p1, hp, :], ps_o[hb][0:64, :], rb[:]
                                )

                        # ---- out projection for this q block ----
                        for ec in range(NDI):
                            ps = ppc.tile([128, 512], F32, tag="ppc")
                            for dl in range(NDC):
                                nc.tensor.matmul(
                                    ps[:],
                                    wo_sb[:, dl, ec * 128 : (ec + 1) * 128],
                                    Xblk[:, dl, :],
                                    start=(dl == 0),
                                    stop=(dl == NDC - 1),
                                )
                            ot = osp.tile([128, 512], F32, tag="ot")
                            nc.scalar.copy(ot[:], ps[:])
                            nc.sync.dma_start(
                                out=outT_d.ap()[ec * 128 : (ec + 1) * 128, qsl],
                                in_=ot[:],
                            )
    nc.compile()
    return nc


# ---------------------------------------------------------------------------
# host side
# ---------------------------------------------------------------------------

_NC_CACHE = {}


def _get_program(variant, reps=1):
    key = (variant, reps)
    if key not in _NC_CACHE:
        _NC_CACHE[key] = build_program(variant, reps)
    return _NC_CACHE[key]


def detect_variant(mask):
    m = np.asarray(mask)
    if (m != 0).all():
        return "ones"
    tril = np.tril(np.ones((S, S), np.int8))
    for b in range(m.shape[0]):
        mb = (m[b] != 0).astype(np.int8)
        if not np.array_equal(mb, tril):
            return "general"
    return "causal"


def make_causal_mask_tiles():
    j = np.arange(4)[:, None, None]
    k = np.arange(128)[None, :, None]
    q = np.arange(512)[None, None, :]
    return (q >= k + 128 * j).astype(np.float32)


def build_in_maps(query, key, value, mask, Wq, bq, Wk, bk, Wv, bv, Wo, bo, variant):
    query = np.asarray(query, np.float32)
    key = np.asarray(key, np.float32)
    value = np.asarray(value, np.float32)
    Wq, Wk, Wv, Wo = (np.asarray(w, np.float32) for w in (Wq, Wk, Wv, Wo))
    bq, bk, bv = (np.asarray(x, np.float32) for x in (bq, bk, bv))

    if variant == "causal":
        mtiles = make_causal_mask_tiles()

    in_maps = []
    for c in range(8):
        b, g = c // 2, c % 2
        gs = slice(g * DL, (g + 1) * DL)
        m = {
            "qT": np.ascontiguousarray(query[b].T),
            "kT": np.ascontiguousarray(key[b].T),
            "vT": np.ascontiguousarray(value[b].T),
            "wq": np.ascontiguousarray(Wq[gs].T),
            "wk": np.ascontiguousarray(Wk[gs].T),
            "wv": np.ascontiguousarray(Wv[gs].T),
            "wo": np.ascontiguousarray(Wo[:, gs].T),
            "bq": np.ascontiguousarray(bq[gs].reshape(NDC, 128).T),
            "bk": np.ascontiguousarray(bk[gs].reshape(NDC, 128).T),
            "bv": np.ascontiguousarray(np.broadcast_to(bv[gs], (128, DL))),
        }
        if variant == "causal":
            m["maskt"] = mtiles
        elif variant == "general":
            m["maskT"] = np.ascontiguousarray(
                (np.asarray(mask[b]) != 0).astype(np.float32).T
            )
        in_maps.append(m)
    return in_maps


def assemble_output(results, bo):
    bo = np.asarray(bo, np.float32)
    out = np.empty((B, S, D), np.float32)
    for b in range(B):
        acc = results[2 * b]["outT"] + results[2 * b + 1]["outT"]
        out[b] = acc.T + bo
    return out


def kernel(query, key, value, mask, Wq, bq, Wk, bk, Wv, bv, Wo, bo):
    variant = detect_variant(np.asarray(mask))
    in_maps = build_in_maps(
        query, key, value, mask, Wq, bq, Wk, bk, Wv, bv, Wo, bo, variant
    )
    nc = _get_program(variant)
    res = run_bass_kernel_spmd(nc, in_maps, core_ids=list(range(8)))
    return assemble_output(res.results, bo)


# revision 9
# speedup vs baseline: 1.6984x; 1.6984x over previous
"""Multi-head attention (B=4, S=2048, D=1024, H=16) on 8 trn2 NeuronCores.

Sharding: core c = 2*b + g handles batch b, head-group g (8 heads, 512 dims).
Q/K/V projections are column-sharded (Megatron), Wo row-sharded; the Wo
partial sums for the two head-groups of each batch are reduced host-side.

Device layout notes:
  - All activations live in "transposed" [feature, seq] layout so every
    matmul has its contraction dim on SBUF partitions.  Host pre-transposes.
  - Matmul operands are FP32R (full PE rate for free dim >= 256, ~1e-4 rel
    precision).  PSUM stays fp32.  DMA'd operands are cast in place by
    GPSIMD copies; engine-produced operands are written as f32r directly.
  - Scores are computed as S^T [k, q] tiles; softmax denominators come from
    an extra ones-column appended to V (PV matmul computes [O^T; rowsum]).
  - exp is issued on [128, 1024] PSUM tiles (2 k-chunks) to amortize ACT
    instruction overhead; no max-subtraction (scores are O(1) by
    construction).  Masking is additive (-1e9) on the PSUM scores pre-exp.
"""

import numpy as np

import concourse.mybir as mybir
import concourse.tile as tile
from concourse import bacc
from concourse.bass_utils import run_bass_kernel_spmd

F32 = mybir.dt.float32
F32R = mybir.dt.float32r
B, S, D, H, DK = 4, 2048, 1024, 16, 64
DL = 512  # dims per head-group (8 heads * 64)
NH = 8  # heads per core
NHP = 4  # head pairs per core
NDI = D // 128  # 8   d_model chunks
NSC = S // 128  # 16  seq chunks (k side)
NSB = S // 512  # 4   seq blocks (q side)
NDC = DL // 128  # 4  local-dim chunks
EXP_GRP = 2  # k-chunks per exp instruction
NEG = -1.0e9



def _emit_qblock(env, _unused, qb):
    nc = env["nc"]
    variant = env["variant"]
    qT_d, mT_d, outT_d = env["qT_d"], env["mT_d"], env["outT_d"]
    stream, raws = env["stream"], env["raws"]
    qbp, xbp, ep, rp, osp, mqp = (
        env["qbp"], env["xbp"], env["ep"], env["rp"], env["osp"], env["mqp"]
    )
    ppq, pss, pso, ppc = env["ppq"], env["pss"], env["pso"], env["ppc"]
    wq_r, wo_r, Ksb, vts = env["wq_r"], env["wo_r"], env["Ksb"], env["vts"]
    bq_sb, mt_sb = env["bq_sb"], env["mt_sb"]

    qsl = slice(qb * 512, (qb + 1) * 512)
    # Q projection for this q block
    qt = stream.tile([128, NDI, 512], F32R, tag="qkstream", name="qt")
    qt_r = qt[:]
    for di in range(NDI):
        raw = raws.tile([128, 512], F32, tag="raw", name=f"qtraw{di}")
        nc.sync.dma_start(out=raw[:], in_=qT_d.ap()[di * 128 : (di + 1) * 128, qsl])
        nc.gpsimd.tensor_copy(qt[:, di, :], raw[:])
    Qblk = qbp.tile([128, NDC, 512], F32R, tag="Qblk", name="Qblk")
    for dc in range(NDC):
        ps = ppq.tile([128, 512], F32, tag="ppq", name="ppq")
        for di in range(NDI):
            nc.tensor.matmul(
                ps[:],
                wq_r[:, di, dc * 128 : (dc + 1) * 128],
                qt_r[:, di, :],
                start=(di == 0),
                stop=(di == NDI - 1),
            )
        nc.vector.tensor_scalar_add(Qblk[:, dc, :], ps[:], env["bq_sb"][:, dc : dc + 1])

    mq_sb = None
    if variant == "general":
        mq_sb = mqp.tile([128, NSC, 512], F32, tag="mq", name="mq")
        nc.sync.dma_start(
            out=mq_sb[:],
            in_=mT_d.ap()[:, qsl].rearrange("(c p) q -> p c q", p=128),
        )

    kept = env["kept_kcs"](qb)
    groups = [kept[i : i + EXP_GRP] for i in range(0, len(kept), EXP_GRP)]
    first, last = kept[0], kept[-1]
    Xblk = xbp.tile([128, NDC, 512], F32R, tag="Xblk", name="Xblk")
    for hp in range(NHP):
        ps_o = {
            0: pso.tile([65, 512], F32, tag="oA", name="psoA"),
            1: pso.tile([65, 512], F32, tag="oB", name="psoB"),
        }
        for grp in groups:
            _emit_group(nc, variant, qb, hp, grp, first, last,
                        pss, ep, Ksb, env["vts"], Qblk, ps_o, mt_sb, mq_sb)
        for hb, (p0, p1) in enumerate(((0, 64), (64, 128))):
            r = rp.tile([1, 512], F32, tag="r", name=f"r{hb}")
            rb = rp.tile([64, 512], F32, tag="rb", name=f"rb{hb}")
            nc.vector.reciprocal(r[:], ps_o[hb][64:65, :])
            nc.gpsimd.partition_broadcast(rb[:], r[0:1, :])
            nc.vector.tensor_mul(Xblk[p0:p1, hp, :], ps_o[hb][0:64, :], rb[:])

    # ---- out projection for this q block ----
    for ec in range(NDI):
        ps = ppc.tile([128, 512], F32, tag="ppc", name="ppc")
        for dl in range(NDC):
            nc.tensor.matmul(
                ps[:],
                wo_r[:, dl, ec * 128 : (ec + 1) * 128],
                Xblk[:, dl, :],
                start=(dl == 0),
                stop=(dl == NDC - 1),
            )
        ot = osp.tile([128, 512], F32, tag="ot", name="ot")
        nc.vector.tensor_copy(ot[:], ps[:])
        nc.sync.dma_start(out=outT_d.ap()[ec * 128 : (ec + 1) * 128, qsl], in_=ot[:])


def _emit_group(nc, variant, qb, hp, grp, first, last, pss, ep, Ksb, vts, Qblk,
                ps_o, mt_sb, mq_sb):
    ng = len(grp)
    ps_s = {
        0: pss.tile([128, EXP_GRP * 512], F32, tag="sA", name="pssA"),
        1: pss.tile([128, EXP_GRP * 512], F32, tag="sB", name="pssB"),
    }
    for hb, (p0, p1) in enumerate(((0, 64), (64, 128))):
        for j, kc in enumerate(grp):
            nc.tensor.matmul(
                ps_s[hb][:, j * 512 : (j + 1) * 512],
                Ksb[p0:p1, hp, kc * 128 : (kc + 1) * 128],
                Qblk[p0:p1, hp, :],
                start=True,
                stop=True,
            )
    # additive masks on PSUM scores (pre-exp)
    for j, kc in enumerate(grp):
        psl = slice(j * 512, (j + 1) * 512)
        if variant == "general":
            for hb in range(2):
                nc.vector.tensor_add(ps_s[hb][:, psl], ps_s[hb][:, psl], mq_sb[:, kc, :])
        elif variant == "causal" and kc >= 4 * qb:
            for hb in range(2):
                nc.vector.tensor_add(
                    ps_s[hb][:, psl], ps_s[hb][:, psl], mt_sb[:, kc - 4 * qb, :]
                )
    es = {}
    for hb in range(2):
        et = ep.tile([128, EXP_GRP * 512], F32R, tag="e", name=f"e{hb}")
        nc.scalar.activation(
            et[:, : ng * 512],
            ps_s[hb][:, : ng * 512],
            mybir.ActivationFunctionType.Exp,
            scale=1.0 / np.sqrt(DK),
        )
        es[hb] = et
    for hb in range(2):
        for j, kc in enumerate(grp):
            h = 2 * hp + hb
            nc.tensor.matmul(
                ps_o[hb][:],
                vts[kc][:, h * 65 : (h + 1) * 65],
                es[hb][:, j * 512 : (j + 1) * 512],
                start=(kc == first),
                stop=(kc == last),
            )


def build_program(variant, reps=1):
    """variant: 'causal' | 'ones' | 'general'"""
    assert variant in ("causal", "ones", "general")
    nc = bacc.Bacc("TRN2", target_bir_lowering=False, debug=False)

    qT_d = nc.dram_tensor("qT", [D, S], F32, kind="ExternalInput")
    kT_d = nc.dram_tensor("kT", [D, S], F32, kind="ExternalInput")
    vT_d = nc.dram_tensor("vT", [D, S], F32, kind="ExternalInput")
    wq_d = nc.dram_tensor("wq", [D, DL], F32, kind="ExternalInput")
    wk_d = nc.dram_tensor("wk", [D, DL], F32, kind="ExternalInput")
    wv_d = nc.dram_tensor("wv", [D, DL], F32, kind="ExternalInput")
    wo_d = nc.dram_tensor("wo", [DL, D], F32, kind="ExternalInput")
    bq_d = nc.dram_tensor("bq", [128, NDC], F32, kind="ExternalInput")
    bk_d = nc.dram_tensor("bk", [128, NDC], F32, kind="ExternalInput")
    bv_d = nc.dram_tensor("bv", [128, DL], F32, kind="ExternalInput")
    if variant == "causal":
        # additive: 0 keep, -1e9 drop; [j, k, q] for diagonal offsets j=0..3
        mt_d = nc.dram_tensor("maskt", [4, 128, 512], F32, kind="ExternalInput")
    elif variant == "general":
        mT_d = nc.dram_tensor("maskT", [S, S], F32, kind="ExternalInput")
    outT_d = nc.dram_tensor("outT", [D, S], F32, kind="ExternalOutput")

    def kept_kcs(qb):
        return list(range(4 * qb + 4)) if variant == "causal" else list(range(NSC))

    with tile.TileContext(nc) as tc:
        for _rep in range(reps):
            with (
                tc.tile_pool(name="persist", bufs=1) as pers,
                tc.tile_pool(name="vt", bufs=1) as vtp,
                tc.tile_pool(name="stream", bufs=1) as stream,
                tc.tile_pool(name="raws", bufs=3) as raws,
                tc.tile_pool(name="qblk", bufs=2) as qbp,
                tc.tile_pool(name="xblk", bufs=2) as xbp,
                tc.tile_pool(name="epool", bufs=3) as ep,
                tc.tile_pool(name="rpool", bufs=2) as rp,
                tc.tile_pool(name="ostage", bufs=3) as osp,
            ):
                # ---- constants ----
                bq_sb = pers.tile([128, NDC], F32, tag="bq")
                bk_sb = pers.tile([128, NDC], F32, tag="bk")
                bv_sb = pers.tile([128, DL], F32, tag="bv")
                ones_sb = pers.tile([128, NH], F32, tag="ones")
                nc.sync.dma_start(out=bq_sb[:], in_=bq_d.ap())
                nc.sync.dma_start(out=bk_sb[:], in_=bk_d.ap())
                nc.sync.dma_start(out=bv_sb[:], in_=bv_d.ap())
                nc.any.memset(ones_sb[:], 1.0)
                if variant == "causal":
                    mt_sb = pers.tile([128, 4, 512], F32, tag="mt")
                    nc.sync.dma_start(
                        out=mt_sb[:], in_=mt_d.ap().rearrange("j p q -> p j q")
                    )
                wo_sb = pers.tile([128, NDC, D], F32R, tag="wo")
                wo_r = wo_sb[:]
                for dl in range(NDC):
                    raw = raws.tile([128, D], F32, tag="raw", name=f"woraw{dl}")
                    nc.sync.dma_start(
                        out=raw[:], in_=wo_d.ap()[dl * 128 : (dl + 1) * 128, :]
                    )
                    nc.gpsimd.tensor_copy(wo_sb[:, dl, :], raw[:])

                Ksb = pers.tile([128, NDC, S], F32R, tag="Ksb")
                vts = [
                    vtp.tile([128, NH * 65], F32R, tag=f"vt{sc}", name=f"vt{sc}")
                    for sc in range(NSC)
                ]

                # ---- K projection:  Ksb[p, dc, s] = (key @ Wk_g.T).T + bk ----
                with (
                    tc.tile_pool(name="wkp", bufs=1) as wkp,
                    tc.tile_pool(name="ppk", bufs=4, space="PSUM") as ppk,
                ):
                    wk_sb = wkp.tile([128, NDI, DL], F32R, tag="wk")
                    wk_r = wk_sb[:]
                    for di in range(NDI):
                        raw = raws.tile([128, DL], F32, tag="raw", name=f"wkraw{di}")
                        nc.sync.dma_start(
                            out=raw[:], in_=wk_d.ap()[di * 128 : (di + 1) * 128, :]
                        )
                        nc.gpsimd.tensor_copy(wk_sb[:, di, :], raw[:])
                    for sb in range(NSB):
                        kt = stream.tile([128, NDI, 512], F32R, tag="qkstream")
                        kt_r = kt[:]
                        for di in range(NDI):
                            raw = raws.tile([128, 512], F32, tag="raw", name=f"ktraw{di}")
                            nc.sync.dma_start(
                                out=raw[:],
                                in_=kT_d.ap()[
                                    di * 128 : (di + 1) * 128,
                                    sb * 512 : (sb + 1) * 512,
                                ],
                            )
                            nc.gpsimd.tensor_copy(kt[:, di, :], raw[:])
                        for dc in range(NDC):
                            ps = ppk.tile([128, 512], F32, tag="ppk")
                            for di in range(NDI):
                                nc.tensor.matmul(
                                    ps[:],
                                    wk_r[:, di, dc * 128 : (dc + 1) * 128],
                                    kt_r[:, di, :],
                                    start=(di == 0),
                                    stop=(di == NDI - 1),
                                )
                            nc.vector.tensor_scalar_add(
                                Ksb[:, dc, sb * 512 : (sb + 1) * 512],
                                ps[:],
                                bk_sb[:, dc : dc + 1],
                            )

                # ---- V projection into [V | 1] tiles ----
                with (
                    tc.tile_pool(name="wvp", bufs=1) as wvp,
                    tc.tile_pool(name="vstream", bufs=2) as vstream,
                    tc.tile_pool(name="ppv", bufs=4, space="PSUM") as ppv,
                ):
                    wv_sb = wvp.tile([128, NDI, DL], F32R, tag="wv")
                    wv_r = wv_sb[:]
                    for di in range(NDI):
                        raw = raws.tile([128, DL], F32, tag="raw", name=f"wvraw{di}")
                        nc.sync.dma_start(
                            out=raw[:], in_=wv_d.ap()[di * 128 : (di + 1) * 128, :]
                        )
                        nc.gpsimd.tensor_copy(wv_sb[:, di, :], raw[:])
                    for sc in range(NSC):
                        vtr = vstream.tile([128, NDI, 128], F32R, tag="vtr")
                        vtr_r = vtr[:]
                        for di in range(NDI):
                            raw = raws.tile([128, 128], F32, tag="raw", name=f"vraw{di}")
                            nc.sync.dma_start(
                                out=raw[:],
                                in_=vT_d.ap()[
                                    di * 128 : (di + 1) * 128,
                                    sc * 128 : (sc + 1) * 128,
                                ],
                            )
                            nc.gpsimd.tensor_copy(vtr[:, di, :], raw[:])
                        ps = ppv.tile([128, DL], F32, tag="ppv")
                        for di in range(NDI):
                            nc.tensor.matmul(
                                ps[:],
                                vtr_r[:, di, :],
                                wv_r[:, di, :],
                                start=(di == 0),
                                stop=(di == NDI - 1),
                            )
                        vt3 = vts[sc][:].rearrange("p (h c) -> p h c", h=NH)
                        nc.vector.tensor_add(
                            vt3[:, :, 0:64],
                            ps[:].rearrange("p (h c) -> p h c", h=NH),
                            bv_sb[:].rearrange("p (h c) -> p h c", h=NH),
                        )
                        nc.vector.tensor_copy(vt3[:, :, 64:65], ones_sb[:].unsqueeze(2))

                # ---- per q-block: Q proj, attention, out proj ----
                with (
                    tc.tile_pool(name="wqp", bufs=1) as wqp,
                    tc.tile_pool(name="mq", bufs=1) as mqp,
                    tc.tile_pool(name="ppq", bufs=1, space="PSUM") as ppq,
                    tc.tile_pool(name="pss", bufs=1, space="PSUM") as pss,
                    tc.tile_pool(name="pso", bufs=1, space="PSUM") as pso,
                    tc.tile_pool(name="ppc", bufs=1, space="PSUM") as ppc,
                ):
                    wq_sb = wqp.tile([128, NDI, DL], F32R, tag="wq")
                    wq_r = wq_sb[:]
                    for di in range(NDI):
                        raw = raws.tile([128, DL], F32, tag="raw", name=f"wqraw{di}")
                        nc.sync.dma_start(
                            out=raw[:], in_=wq_d.ap()[di * 128 : (di + 1) * 128, :]
                        )
                        nc.gpsimd.tensor_copy(wq_sb[:, di, :], raw[:])
                    env = dict(
                        nc=nc, variant=variant, kept_kcs=kept_kcs,
                        qT_d=qT_d, mT_d=(mT_d if variant == "general" else None),
                        outT_d=outT_d,
                        stream=stream, raws=raws, qbp=qbp, xbp=xbp, ep=ep,
                        rp=rp, osp=osp, mqp=mqp,
                        ppq=ppq, pss=pss, pso=pso, ppc=ppc,
                        wq_r=wq_r, wo_r=wo_r, Ksb=Ksb, vts=vts,
                        bq_sb=bq_sb,
                        mt_sb=(mt_sb if variant == "causal" else None),
                    )
                    for qb in range(NSB):
                        _emit_qblock(env, locals(), qb)
    nc.compile()
    return nc


# ---------------------------------------------------------------------------
# host side
# ---------------------------------------------------------------------------

_NC_CACHE = {}


def _get_program(variant, reps=1):
    key = (variant, reps)
    if key not in _NC_CACHE:
        _NC_CACHE[key] = build_program(variant, reps)
    return _NC_CACHE[key]


def detect_variant(mask):
    m = np.asarray(mask)
    if (m != 0).all():
        return "ones"
    tril = np.tril(np.ones((S, S), np.int8))
    for b in range(m.shape[0]):
        mb = (m[b] != 0).astype(np.int8)
        if not np.array_equal(mb, tril):
            return "general"
    return "causal"


def make_causal_mask_tiles():
    j = np.arange(4)[:, None, None]
    k = np.arange(128)[None, :, None]
    q = np.arange(512)[None, None, :]
    return np.where(q >= k + 128 * j, 0.0, NEG).astype(np.float32)


def build_in_maps(query, key, value, mask, Wq, bq, Wk, bk, Wv, bv, Wo, bo, variant):
    query = np.asarray(query, np.float32)
    key = np.asarray(key, np.float32)
    value = np.asarray(value, np.float32)
    Wq, Wk, Wv, Wo = (np.asarray(w, np.float32) for w in (Wq, Wk, Wv, Wo))
    bq, bk, bv = (np.asarray(x, np.float32) for x in (bq, bk, bv))

    if variant == "causal":
        mtiles = make_causal_mask_tiles()

    in_maps = []
    for c in range(8):
        b, g = c // 2, c % 2
        gs = slice(g * DL, (g + 1) * DL)
        m = {
            "qT": np.ascontiguousarray(query[b].T),
            "kT": np.ascontiguousarray(key[b].T),
            "vT": np.ascontiguousarray(value[b].T),
            "wq": np.ascontiguousarray(Wq[gs].T),
            "wk": np.ascontiguousarray(Wk[gs].T),
            "wv": np.ascontiguousarray(Wv[gs].T),
            "wo": np.ascontiguousarray(Wo[:, gs].T),
            "bq": np.ascontiguousarray(bq[gs].reshape(NDC, 128).T),
            "bk": np.ascontiguousarray(bk[gs].reshape(NDC, 128).T),
            "bv": np.ascontiguousarray(np.broadcast_to(bv[gs], (128, DL))),
        }
        if variant == "causal":
            m["maskt"] = mtiles
        elif variant == "general":
            m["maskT"] = np.ascontiguousarray(
                np.where(np.asarray(mask[b]) != 0, 0.0, NEG).astype(np.float32).T
            )
        in_maps.append(m)
    return in_maps


def assemble_output(results, bo):
    bo = np.asarray(bo, np.float32)
    out = np.empty((B, S, D), np.float32)
    for b in range(B):
        acc = results[2 * b]["outT"] + results[2 * b + 1]["outT"]
        out[b] = acc.T + bo
    return out


def kernel(query, key, value, mask, Wq, bq, Wk, bk, Wv, bv, Wo, bo):
    variant = detect_variant(np.asarray(mask))
    in_maps = build_in_maps(
        query, key, value, mask, Wq, bq, Wk, bk, Wv, bv, Wo, bo, variant
    )
    nc = _get_program(variant)
    res = run_bass_kernel_spmd(nc, in_maps, core_ids=list(range(8)))
    return assemble_output(res.results, bo)


# revision 11
# speedup vs baseline: 1.7832x; 1.0499x over previous
"""Multi-head attention (B=4, S=2048, D=1024, H=16) on 8 trn2 NeuronCores.

Sharding: core c = 2*b + g handles batch b, head-group g (8 heads, 512 dims).
Q/K/V projections are column-sharded (Megatron), Wo row-sharded; the Wo
partial sums for the two head-groups of each batch are reduced host-side.

Device layout notes:
  - All activations live in "transposed" [feature, seq] layout so every
    matmul has its contraction dim on SBUF partitions.  Host pre-transposes.
  - Matmul operands are FP32R (full PE rate for free dim >= 256, ~1e-4 rel
    precision).  PSUM stays fp32.  DMA'd operands are cast in place by
    GPSIMD copies; engine-produced operands are written as f32r directly.
  - Scores are computed as S^T [k, q] tiles; softmax denominators come from
    an extra ones-column appended to V (PV matmul computes [O^T; rowsum]).
  - exp is issued on [128, 1024] PSUM tiles (2 k-chunks) to amortize ACT
    instruction overhead; no max-subtraction (scores are O(1) by
    construction).  Masking is additive (-1e9) on the PSUM scores pre-exp.
"""

import numpy as np

import concourse.mybir as mybir
import concourse.tile as tile
from concourse import bacc
from concourse.bass_utils import run_bass_kernel_spmd

F32 = mybir.dt.float32
F32R = mybir.dt.float32r
B, S, D, H, DK = 4, 2048, 1024, 16, 64
DL = 512  # dims per head-group (8 heads * 64)
NH = 8  # heads per core
NHP = 4  # head pairs per core
NDI = D // 128  # 8   d_model chunks
NSC = S // 128  # 16  seq chunks (k side)
NSB = S // 512  # 4   seq blocks (q side)
NDC = DL // 128  # 4  local-dim chunks
EXP_GRP = 2  # k-chunks per exp instruction
NEG = -1.0e9



def _emit_qblock(env, _unused, qb):
    nc = env["nc"]
    variant = env["variant"]
    qT_d, mT_d, outT_d = env["qT_d"], env["mT_d"], env["outT_d"]
    stream, raws = env["stream"], env["raws"]
    qbp, xbp, ep, rp, osp, mqp = (
        env["qbp"], env["xbp"], env["ep"], env["rp"], env["osp"], env["mqp"]
    )
    pss, pso = env["pss"], env["pso"]
    wq_r, wo_r, Ksb, vts = env["wq_r"], env["wo_r"], env["Ksb"], env["vts"]
    bq_sb, mt_sb = env["bq_sb"], env["mt_sb"]

    qsl = slice(qb * 512, (qb + 1) * 512)
    # Q projection for this q block
    qt = stream.tile([128, NDI, 512], F32R, tag="qkstream", name="qt")
    qt_r = qt[:]
    for di in range(NDI):
        raw = raws.tile([128, 512], F32, tag="raw", name=f"qtraw{di}")
        nc.sync.dma_start(out=raw[:], in_=qT_d.ap()[di * 128 : (di + 1) * 128, qsl])
        nc.gpsimd.tensor_copy(qt[:, di, :], raw[:])
    Qblk = qbp.tile([128, NDC, 512], F32R, tag="Qblk", name="Qblk")
    for dc in range(NDC):
        ps = pss.tile([128, 512], F32, tag="s", name="ppq")
        for di in range(NDI):
            nc.tensor.matmul(
                ps[:],
                wq_r[:, di, dc * 128 : (dc + 1) * 128],
                qt_r[:, di, :],
                start=(di == 0),
                stop=(di == NDI - 1),
            )
        nc.vector.tensor_scalar_add(Qblk[:, dc, :], ps[:], env["bq_sb"][:, dc : dc + 1])

    mq_sb = None
    if variant == "general":
        mq_sb = mqp.tile([128, NSC, 512], F32, tag="mq", name="mq")
        nc.sync.dma_start(
            out=mq_sb[:],
            in_=mT_d.ap()[:, qsl].rearrange("(c p) q -> p c q", p=128),
        )

    kept = env["kept_kcs"](qb)
    groups = [kept[i : i + EXP_GRP] for i in range(0, len(kept), EXP_GRP)]
    first, last = kept[0], kept[-1]
    skip = env["skip"]
    Xblk = xbp.tile([128, NDC, 512], F32R, tag="Xblk", name="Xblk")
    for hp in range(NHP if "attn" not in skip else 0):
        ps_o = {
            0: pso.tile([65, 512], F32, tag="oA", name="psoA"),
            1: pso.tile([65, 512], F32, tag="oB", name="psoB"),
        }
        for grp in groups:
            _emit_group(nc, variant, qb, hp, grp, first, last,
                        pss, ep, Ksb, env["vts"], Qblk, ps_o, mt_sb, mq_sb,
                        skip_pv=("pv" in skip))
        if "pv" in skip:
            continue
        for hb, (p0, p1) in enumerate(((0, 64), (64, 128))):
            r = rp.tile([1, 512], F32, tag="r", name=f"r{hb}")
            rb = rp.tile([64, 512], F32, tag="rb", name=f"rb{hb}")
            nc.vector.reciprocal(r[:], ps_o[hb][64:65, :])
            nc.gpsimd.partition_broadcast(rb[:], r[0:1, :])
            nc.vector.tensor_mul(Xblk[p0:p1, hp, :], ps_o[hb][0:64, :], rb[:])

    # ---- out projection for this q block ----
    if "out" in skip or "pv" in skip or "attn" in skip:
        return
    for ec in range(NDI):
        ps = pss.tile([128, 512], F32, tag="s", name="ppc")
        for dl in range(NDC):
            nc.tensor.matmul(
                ps[:],
                wo_r[:, dl, ec * 128 : (ec + 1) * 128],
                Xblk[:, dl, :],
                start=(dl == 0),
                stop=(dl == NDC - 1),
            )
        ot = osp.tile([128, 512], F32, tag="ot", name="ot")
        nc.vector.tensor_copy(ot[:], ps[:])
        nc.sync.dma_start(out=outT_d.ap()[ec * 128 : (ec + 1) * 128, qsl], in_=ot[:])


def _emit_group(nc, variant, qb, hp, grp, first, last, pss, ep, Ksb, vts, Qblk,
                ps_o, mt_sb, mq_sb, skip_pv=False):
    ng = len(grp)
    ps_s = {
        0: pss.tile([128, EXP_GRP * 512], F32, tag="s", name="pssA"),
        1: pss.tile([128, EXP_GRP * 512], F32, tag="s", name="pssB"),
    }
    for hb, (p0, p1) in enumerate(((0, 64), (64, 128))):
        for j, kc in enumerate(grp):
            nc.tensor.matmul(
                ps_s[hb][:, j * 512 : (j + 1) * 512],
                Ksb[p0:p1, hp, kc * 128 : (kc + 1) * 128],
                Qblk[p0:p1, hp, :],
                start=True,
                stop=True,
            )
    # additive masks on PSUM scores (pre-exp)
    for j, kc in enumerate(grp):
        psl = slice(j * 512, (j + 1) * 512)
        if variant == "general":
            for hb in range(2):
                nc.vector.tensor_add(ps_s[hb][:, psl], ps_s[hb][:, psl], mq_sb[:, kc, :])
        elif variant == "causal" and kc >= 4 * qb:
            for hb in range(2):
                nc.vector.tensor_add(
                    ps_s[hb][:, psl], ps_s[hb][:, psl], mt_sb[:, kc - 4 * qb, :]
                )
    es = {}
    for hb in range(2):
        et = ep.tile([128, EXP_GRP * 512], F32R, tag="e", name=f"e{hb}")
        nc.scalar.activation(
            et[:, : ng * 512],
            ps_s[hb][:, : ng * 512],
            mybir.ActivationFunctionType.Exp,
            scale=1.0 / np.sqrt(DK),
        )
        es[hb] = et
    if skip_pv:
        return
    for hb in range(2):
        for j, kc in enumerate(grp):
            h = 2 * hp + hb
            nc.tensor.matmul(
                ps_o[hb][:],
                vts[kc][:, h * 65 : (h + 1) * 65],
                es[hb][:, j * 512 : (j + 1) * 512],
                start=(kc == first),
                stop=(kc == last),
            )


def build_program(variant, reps=1, skip=()):
    """variant: 'causal' | 'ones' | 'general'; skip: subset of {"qk","v","attn","pv","out"} (timing ablations)"""
    assert variant in ("causal", "ones", "general")
    nc = bacc.Bacc("TRN2", target_bir_lowering=False, debug=False)

    qT_d = nc.dram_tensor("qT", [D, S], F32, kind="ExternalInput")
    kT_d = nc.dram_tensor("kT", [D, S], F32, kind="ExternalInput")
    vT_d = nc.dram_tensor("vT", [D, S], F32, kind="ExternalInput")
    wq_d = nc.dram_tensor("wq", [D, DL], F32, kind="ExternalInput")
    wk_d = nc.dram_tensor("wk", [D, DL], F32, kind="ExternalInput")
    wv_d = nc.dram_tensor("wv", [D, DL], F32, kind="ExternalInput")
    wo_d = nc.dram_tensor("wo", [DL, D], F32, kind="ExternalInput")
    bq_d = nc.dram_tensor("bq", [128, NDC], F32, kind="ExternalInput")
    bk_d = nc.dram_tensor("bk", [128, NDC], F32, kind="ExternalInput")
    bv_d = nc.dram_tensor("bv", [128, DL], F32, kind="ExternalInput")
    if variant == "causal":
        # additive: 0 keep, -1e9 drop; [j, k, q] for diagonal offsets j=0..3
        mt_d = nc.dram_tensor("maskt", [4, 128, 512], F32, kind="ExternalInput")
    elif variant == "general":
        mT_d = nc.dram_tensor("maskT", [S, S], F32, kind="ExternalInput")
    outT_d = nc.dram_tensor("outT", [D, S], F32, kind="ExternalOutput")

    def kept_kcs(qb):
        return list(range(4 * qb + 4)) if variant == "causal" else list(range(NSC))

    with tile.TileContext(nc) as tc:
        for _rep in range(reps):
            with (
                tc.tile_pool(name="persist", bufs=1) as pers,
                tc.tile_pool(name="vt", bufs=1) as vtp,
                tc.tile_pool(name="stream", bufs=2) as stream,
                tc.tile_pool(name="raws", bufs=3) as raws,
                tc.tile_pool(name="qblk", bufs=2) as qbp,
                tc.tile_pool(name="xblk", bufs=2) as xbp,
                tc.tile_pool(name="epool", bufs=3) as ep,
                tc.tile_pool(name="rpool", bufs=1) as rp,
                tc.tile_pool(name="ostage", bufs=2) as osp,
            ):
                # ---- constants ----
                bq_sb = pers.tile([128, NDC], F32, tag="bq")
                bk_sb = pers.tile([128, NDC], F32, tag="bk")
                bv_sb = pers.tile([128, DL], F32, tag="bv")
                ones_sb = pers.tile([128, NH], F32, tag="ones")
                nc.sync.dma_start(out=bq_sb[:], in_=bq_d.ap())
                nc.sync.dma_start(out=bk_sb[:], in_=bk_d.ap())
                nc.sync.dma_start(out=bv_sb[:], in_=bv_d.ap())
                nc.any.memset(ones_sb[:], 1.0)
                if variant == "causal":
                    mt_sb = pers.tile([128, 4, 512], F32, tag="mt")
                    nc.sync.dma_start(
                        out=mt_sb[:], in_=mt_d.ap().rearrange("j p q -> p j q")
                    )
                wo_sb = pers.tile([128, NDC, D], F32R, tag="wo")
                wo_r = wo_sb[:]
                for dl in range(NDC):
                    for hh in range(2):
                        raw = raws.tile(
                            [128, 512], F32, tag="raw", name=f"woraw{dl}_{hh}"
                        )
                        nc.sync.dma_start(
                            out=raw[:],
                            in_=wo_d.ap()[
                                dl * 128 : (dl + 1) * 128,
                                hh * 512 : (hh + 1) * 512,
                            ],
                        )
                        nc.gpsimd.tensor_copy(
                            wo_sb[:, dl, hh * 512 : (hh + 1) * 512], raw[:]
                        )

                Ksb = pers.tile([128, NDC, S], F32R, tag="Ksb")
                vts = [
                    vtp.tile([128, NH * 65], F32R, tag=f"vt{sc}", name=f"vt{sc}")
                    for sc in range(NSC)
                ]

                # ---- K projection:  Ksb[p, dc, s] = (key @ Wk_g.T).T + bk ----
                if "qk" not in skip:
                  with (
                    tc.tile_pool(name="wkp", bufs=1) as wkp,
                    tc.tile_pool(name="ppk", bufs=4, space="PSUM") as ppk,
                ):
                    wk_sb = wkp.tile([128, NDI, DL], F32R, tag="wk")
                    wk_r = wk_sb[:]
                    for di in range(NDI):
                        raw = raws.tile([128, DL], F32, tag="raw", name=f"wkraw{di}")
                        nc.sync.dma_start(
                            out=raw[:], in_=wk_d.ap()[di * 128 : (di + 1) * 128, :]
                        )
                        nc.gpsimd.tensor_copy(wk_sb[:, di, :], raw[:])
                    for sb in range(NSB):
                        kt = stream.tile([128, NDI, 512], F32R, tag="qkstream")
                        kt_r = kt[:]
                        for di in range(NDI):
                            raw = raws.tile([128, 512], F32, tag="raw", name=f"ktraw{di}")
                            nc.sync.dma_start(
                                out=raw[:],
                                in_=kT_d.ap()[
                                    di * 128 : (di + 1) * 128,
                                    sb * 512 : (sb + 1) * 512,
                                ],
                            )
                            nc.gpsimd.tensor_copy(kt[:, di, :], raw[:])
                        for dc in range(NDC):
                            ps = ppk.tile([128, 512], F32, tag="ppk")
                            for di in range(NDI):
                                nc.tensor.matmul(
                                    ps[:],
                                    wk_r[:, di, dc * 128 : (dc + 1) * 128],
                                    kt_r[:, di, :],
                                    start=(di == 0),
                                    stop=(di == NDI - 1),
                                )
                            nc.vector.tensor_scalar_add(
                                Ksb[:, dc, sb * 512 : (sb + 1) * 512],
                                ps[:],
                                bk_sb[:, dc : dc + 1],
                            )

                # ---- V projection into [V | 1] tiles ----
                if "v" not in skip:
                  with (
                    tc.tile_pool(name="wvp", bufs=1) as wvp,
                    tc.tile_pool(name="vstream", bufs=2) as vstream,
                    tc.tile_pool(name="ppv", bufs=4, space="PSUM") as ppv,
                ):
                    wv_sb = wvp.tile([128, NDI, DL], F32R, tag="wv")
                    wv_r = wv_sb[:]
                    for di in range(NDI):
                        raw = raws.tile([128, DL], F32, tag="raw", name=f"wvraw{di}")
                        nc.sync.dma_start(
                            out=raw[:], in_=wv_d.ap()[di * 128 : (di + 1) * 128, :]
                        )
                        nc.gpsimd.tensor_copy(wv_sb[:, di, :], raw[:])
                    for sc in range(NSC):
                        vtr = vstream.tile([128, NDI, 128], F32R, tag="vtr")
                        vtr_r = vtr[:]
                        for di in range(NDI):
                            raw = raws.tile([128, 128], F32, tag="raw", name=f"vraw{di}")
                            nc.sync.dma_start(
                                out=raw[:],
                                in_=vT_d.ap()[
                                    di * 128 : (di + 1) * 128,
                                    sc * 128 : (sc + 1) * 128,
                                ],
                            )
                            nc.gpsimd.tensor_copy(vtr[:, di, :], raw[:])
                        ps = ppv.tile([128, DL], F32, tag="ppv")
                        for di in range(NDI):
                            nc.tensor.matmul(
                                ps[:],
                                vtr_r[:, di, :],
                                wv_r[:, di, :],
                                start=(di == 0),
                                stop=(di == NDI - 1),
                            )
                        vt3 = vts[sc][:].rearrange("p (h c) -> p h c", h=NH)
                        nc.vector.tensor_add(
                            vt3[:, :, 0:64],
                            ps[:].rearrange("p (h c) -> p h c", h=NH),
                            bv_sb[:].rearrange("p (h c) -> p h c", h=NH),
                        )
                        nc.vector.tensor_copy(vt3[:, :, 64:65], ones_sb[:].unsqueeze(2))

                # ---- per q-block: Q proj, attention, out proj ----
                with (
                    tc.tile_pool(name="wqp", bufs=1) as wqp,
                    tc.tile_pool(name="mq", bufs=1) as mqp,
                    tc.tile_pool(name="pss", bufs=3, space="PSUM") as pss,
                    tc.tile_pool(name="pso", bufs=1, space="PSUM") as pso,
                ):
                    wq_sb = wqp.tile([128, NDI, DL], F32R, tag="wq")
                    wq_r = wq_sb[:]
                    for di in range(NDI):
                        raw = raws.tile([128, DL], F32, tag="raw", name=f"wqraw{di}")
                        nc.sync.dma_start(
                            out=raw[:], in_=wq_d.ap()[di * 128 : (di + 1) * 128, :]
                        )
                        nc.gpsimd.tensor_copy(wq_sb[:, di, :], raw[:])
                    env = dict(
                        skip=skip,
                        nc=nc, variant=variant, kept_kcs=kept_kcs,
                        qT_d=qT_d, mT_d=(mT_d if variant == "general" else None),
                        outT_d=outT_d,
                        stream=stream, raws=raws, qbp=qbp, xbp=xbp, ep=ep,
                        rp=rp, osp=osp, mqp=mqp,
                        pss=pss, pso=pso,
                        wq_r=wq_r, wo_r=wo_r, Ksb=Ksb, vts=vts,
                        bq_sb=bq_sb,
                        mt_sb=(mt_sb if variant == "causal" else None),
                    )
                    for qb in range(NSB):
                        _emit_qblock(env, locals(), qb)
    nc.compile()
    return nc


# ---------------------------------------------------------------------------
# host side
# ---------------------------------------------------------------------------

_NC_CACHE = {}


def _get_program(variant, reps=1):
    key = (variant, reps)
    if key not in _NC_CACHE:
        _NC_CACHE[key] = build_program(variant, reps)
    return _NC_CACHE[key]


def detect_variant(mask):
    m = np.asarray(mask)
    if (m != 0).all():
        return "ones"
    tril = np.tril(np.ones((S, S), np.int8))
    for b in range(m.shape[0]):
        mb = (m[b] != 0).astype(np.int8)
        if not np.array_equal(mb, tril):
            return "general"
    return "causal"


def make_causal_mask_tiles():
    j = np.arange(4)[:, None, None]
    k = np.arange(128)[None, :, None]
    q = np.arange(512)[None, None, :]
    return np.where(q >= k + 128 * j, 0.0, NEG).astype(np.float32)


def build_in_maps(query, key, value, mask, Wq, bq, Wk, bk, Wv, bv, Wo, bo, variant):
    query = np.asarray(query, np.float32)
    key = np.asarray(key, np.float32)
    value = np.asarray(value, np.float32)
    Wq, Wk, Wv, Wo = (np.asarray(w, np.float32) for w in (Wq, Wk, Wv, Wo))
    bq, bk, bv = (np.asarray(x, np.float32) for x in (bq, bk, bv))

    if variant == "causal":
        mtiles = make_causal_mask_tiles()

    in_maps = []
    for c in range(8):
        b, g = c // 2, c % 2
        gs = slice(g * DL, (g + 1) * DL)
        m = {
            "qT": np.ascontiguousarray(query[b].T),
            "kT": np.ascontiguousarray(key[b].T),
            "vT": np.ascontiguousarray(value[b].T),
            "wq": np.ascontiguousarray(Wq[gs].T),
            "wk": np.ascontiguousarray(Wk[gs].T),
            "wv": np.ascontiguousarray(Wv[gs].T),
            "wo": np.ascontiguousarray(Wo[:, gs].T),
            "bq": np.ascontiguousarray(bq[gs].reshape(NDC, 128).T),
            "bk": np.ascontiguousarray(bk[gs].reshape(NDC, 128).T),
            "bv": np.ascontiguousarray(np.broadcast_to(bv[gs], (128, DL))),
        }
        if variant == "causal":
            m["maskt"] = mtiles
        elif variant == "general":
            m["maskT"] = np.ascontiguousarray(
                np.where(np.asarray(mask[b]) != 0, 0.0, NEG).astype(np.float32).T
            )
        in_maps.append(m)
    return in_maps


def assemble_output(results, bo):
    bo = np.asarray(bo, np.float32)
    out = np.empty((B, S, D), np.float32)
    for b in range(B):
        acc = results[2 * b]["outT"] + results[2 * b + 1]["outT"]
        out[b] = acc.T + bo
    return out


def kernel(query, key, value, mask, Wq, bq, Wk, bk, Wv, bv, Wo, bo):
    variant = detect_variant(np.asarray(mask))
    in_maps = build_in_maps(
        query, key, value, mask, Wq, bq, Wk, bk, Wv, bv, Wo, bo, variant
    )
    nc = _get_program(variant)
    res = run_bass_kernel_spmd(nc, in_maps, core_ids=list(range(8)))
    return assemble_output(res.results, bo)


# revision 17
# speedup vs baseline: 2.0987x; 1.1769x over previous
"""Multi-head attention (B=4, S=2048, D=1024, H=16) on 8 trn2 NeuronCores.

Sharding: core c = 2*b + g handles batch b, head-group g (8 heads, 512 dims).
Q/K/V projections are column-sharded (Megatron), Wo row-sharded; the Wo
partial sums for the two head-groups of each batch are reduced host-side.

Device layout notes:
  - All activations live in "transposed" [feature, seq] layout so every
    matmul has its contraction dim on SBUF partitions.  Host pre-transposes.
  - Matmul operands are FP32R (full PE rate for free dim >= 256, ~1e-4 rel
    precision).  PSUM stays fp32.  DMA'd operands are cast in place by
    GPSIMD copies; engine-produced operands are written as f32r directly.
  - Scores are computed as S^T [k, q] tiles; softmax denominators come from
    an extra ones-column appended to V (PV matmul computes [O^T; rowsum]).
  - exp is issued on [128, 1024] PSUM tiles (2 k-chunks) to amortize ACT
    instruction overhead; no max-subtraction (scores are O(1) by
    construction).  Masking is additive (-1e9) on the PSUM scores pre-exp.
"""

import numpy as np

import concourse.mybir as mybir
import concourse.tile as tile
from concourse import bacc
from concourse.bass_utils import run_bass_kernel_spmd

F32 = mybir.dt.float32
F32R = mybir.dt.float32r
B, S, D, H, DK = 4, 2048, 1024, 16, 64
DL = 512  # dims per head-group (8 heads * 64)
NH = 8  # heads per core
NHP = 4  # head pairs per core
NDI = D // 128  # 8   d_model chunks
NSC = S // 128  # 16  seq chunks (k side)
NSB = S // 512  # 4   seq blocks (q side)
NDC = DL // 128  # 4  local-dim chunks
EXP_GRP = 2  # k-chunks per exp instruction
NEG = -1.0e9



def _emit_qblock(env, _unused, qb):
    nc = env["nc"]
    variant = env["variant"]
    qT_d, mT_d, outT_d = env["qT_d"], env["mT_d"], env["outT_d"]
    stream, raws = env["stream"], env["raws"]
    qbp, xbp, ep, rp, osp, mqp = (
        env["qbp"], env["xbp"], env["ep"], env["rp"], env["osp"], env["mqp"]
    )
    pss, pso = env["pss"], env["pso"]
    wq_r, wo_r, Ksb, vts = env["wq_r"], env["wo_r"], env["Ksb"], env["vts"]
    bq_sb, mt_sb = env["bq_sb"], env["mt_sb"]

    qsl = slice(qb * 512, (qb + 1) * 512)
    # Q projection for this q block
    qt = stream.tile([128, NDI, 512], F32R, tag="qkstream", name="qt")
    qt_r = qt[:]
    for di in range(NDI):
        raw = raws.tile([128, 512], F32, tag="raw", name=f"qtraw{di}")
        nc.sync.dma_start(out=raw[:], in_=qT_d.ap()[di * 128 : (di + 1) * 128, qsl])
        nc.gpsimd.tensor_copy(qt[:, di, :], raw[:])
    Qblk = qbp.tile([128, NDC, 512], F32R, tag="Qblk", name="Qblk")
    for dc in range(NDC):
        ps = pss.tile([128, 512], F32, tag="s", name="ppq")
        for di in range(NDI):
            nc.tensor.matmul(
                ps[:],
                wq_r[:, di, dc * 128 : (dc + 1) * 128],
                qt_r[:, di, :],
                start=(di == 0),
                stop=(di == NDI - 1),
            )
        nc.vector.tensor_scalar_add(Qblk[:, dc, :], ps[:], env["bq_sb"][:, dc : dc + 1])

    mq_sb = None
    if variant == "general":
        mq_sb = mqp.tile([128, NSC, 512], F32, tag="mq", name="mq")
        nc.sync.dma_start(
            out=mq_sb[:],
            in_=mT_d.ap()[:, qsl].rearrange("(c p) q -> p c q", p=128),
        )

    kept = env["kept_kcs"](qb)
    groups = [kept[i : i + EXP_GRP] for i in range(0, len(kept), EXP_GRP)]
    first, last = kept[0], kept[-1]
    skip = env["skip"]
    Xblk = None
    if not ({"pv", "exp", "attn"} & set(skip)):
        Xblk = xbp.tile([128, NDC, 512], F32R, tag="Xblk", name="Xblk")
    for hp in range(NHP if "attn" not in skip else 0):
        ps_o = {
            0: pso.tile([65, 512], F32, tag="oA", name="psoA"),
            1: pso.tile([65, 512], F32, tag="oB", name="psoB"),
        }
        for grp in groups:
            _emit_group(nc, variant, qb, hp, grp, first, last,
                        pss, ep, Ksb, env["vts"], Qblk, ps_o, mt_sb, mq_sb,
                        skip_pv=("pv" in skip or "exp" in skip),
                        skip_mask=("nomask" in skip or "exp" in skip),
                        skip_exp=("exp" in skip))
        if "pv" in skip or "exp" in skip:
            continue
        for hb, (p0, p1) in enumerate(((0, 64), (64, 128))):
            r = rp.tile([1, 512], F32, tag="r", name=f"r{hb}")
            rb = rp.tile([64, 512], F32, tag="rb", name=f"rb{hb}")
            nc.vector.reciprocal(r[:], ps_o[hb][64:65, :])
            nc.gpsimd.partition_broadcast(rb[:], r[0:1, :])
            nc.vector.tensor_mul(Xblk[p0:p1, hp, :], ps_o[hb][0:64, :], rb[:])

    # ---- out projection for this q block ----
    if {"out", "pv", "attn", "exp"} & set(skip):
        return
    for ec in range(NDI):
        ps = pss.tile([128, 512], F32, tag="s", name="ppc")
        for dl in range(NDC):
            nc.tensor.matmul(
                ps[:],
                wo_r[:, dl, ec * 128 : (ec + 1) * 128],
                Xblk[:, dl, :],
                start=(dl == 0),
                stop=(dl == NDC - 1),
            )
        ot = osp.tile([128, 512], F32, tag="ot", name="ot")
        nc.vector.tensor_copy(ot[:], ps[:])
        nc.sync.dma_start(out=outT_d.ap()[ec * 128 : (ec + 1) * 128, qsl], in_=ot[:])


def _emit_group(nc, variant, qb, hp, grp, first, last, pss, ep, Ksb, vts, Qblk,
                ps_o, mt_sb, mq_sb, skip_pv=False, skip_mask=False, skip_exp=False):
    ng = len(grp)
    ps_s = {
        0: pss.tile([128, EXP_GRP * 512], F32, tag="s", name="pssA"),
        1: pss.tile([128, EXP_GRP * 512], F32, tag="s", name="pssB"),
    }
    for hb, (p0, p1) in enumerate(((0, 64), (64, 128))):
        for j, kc in enumerate(grp):
            nc.tensor.matmul(
                ps_s[hb][:, j * 512 : (j + 1) * 512],
                Ksb[p0:p1, hp, kc * 128 : (kc + 1) * 128],
                Qblk[p0:p1, hp, :],
                start=True,
                stop=True,
            )
    # general variant: additive masks on PSUM scores (pre-exp)
    if variant == "general" and not skip_mask:
        for j, kc in enumerate(grp):
            psl = slice(j * 512, (j + 1) * 512)
            for hb in range(2):
                nc.vector.tensor_add(ps_s[hb][:, psl], ps_s[hb][:, psl], mq_sb[:, kc, :])
    if skip_exp:
        return
    es = {}
    for hb in range(2):
        et = ep.tile([128, EXP_GRP * 512], F32R, tag="e", name=f"e{hb}")
        nc.scalar.activation(
            et[:, : ng * 512],
            ps_s[hb][:, : ng * 512],
            mybir.ActivationFunctionType.Exp,
            scale=1.0 / np.sqrt(DK),
        )
        es[hb] = et
    # causal: multiplicative zeroing post-exp on SBUF (off the ACT chain)
    if variant == "causal" and not skip_mask:
        for j, kc in enumerate(grp):
            if kc >= 4 * qb:
                psl = slice(j * 512, (j + 1) * 512)
                for hb in range(2):
                    nc.vector.tensor_mul(
                        es[hb][:, psl], es[hb][:, psl], mt_sb[:, kc - 4 * qb, :]
                    )
    if skip_pv:
        return
    for hb in range(2):
        for j, kc in enumerate(grp):
            h = 2 * hp + hb
            nc.tensor.matmul(
                ps_o[hb][:],
                vts[kc][:, h * 65 : (h + 1) * 65],
                es[hb][:, j * 512 : (j + 1) * 512],
                start=(kc == first),
                stop=(kc == last),
            )


def build_program(variant, reps=1, skip=()):
    """variant: 'causal' | 'ones' | 'general'; skip: subset of {"qk","v","attn","pv","out"} (timing ablations)"""
    assert variant in ("causal", "ones", "general")
    nc = bacc.Bacc("TRN2", target_bir_lowering=False, debug=False)

    qT_d = nc.dram_tensor("qT", [D, S], F32, kind="ExternalInput")
    kT_d = nc.dram_tensor("kT", [D, S], F32, kind="ExternalInput")
    vT_d = nc.dram_tensor("vT", [D, S], F32, kind="ExternalInput")
    wq_d = nc.dram_tensor("wq", [D, DL], F32, kind="ExternalInput")
    wk_d = nc.dram_tensor("wk", [D, DL], F32, kind="ExternalInput")
    wv_d = nc.dram_tensor("wv", [D, DL], F32, kind="ExternalInput")
    wo_d = nc.dram_tensor("wo", [DL, D], F32, kind="ExternalInput")
    bq_d = nc.dram_tensor("bq", [128, NDC], F32, kind="ExternalInput")
    bk_d = nc.dram_tensor("bk", [128, NDC], F32, kind="ExternalInput")
    bv_d = nc.dram_tensor("bv", [128, DL], F32, kind="ExternalInput")
    if variant == "causal":
        # additive: 0 keep, -1e9 drop; [j, k, q] for diagonal offsets j=0..3
        mt_d = nc.dram_tensor("maskt", [4, 128, 512], F32, kind="ExternalInput")
    elif variant == "general":
        mT_d = nc.dram_tensor("maskT", [S, S], F32, kind="ExternalInput")
    outT_d = nc.dram_tensor("outT", [D, S], F32, kind="ExternalOutput")

    def kept_kcs(qb):
        return list(range(4 * qb + 4)) if variant == "causal" else list(range(NSC))

    with tile.TileContext(nc) as tc:
        for _rep in range(reps):
            with (
                tc.tile_pool(name="persist", bufs=1) as pers,
                tc.tile_pool(name="vt", bufs=1) as vtp,
                tc.tile_pool(name="stream", bufs=2) as stream,
                tc.tile_pool(name="raws", bufs=3) as raws,
                tc.tile_pool(name="qblk", bufs=2) as qbp,
                tc.tile_pool(name="xblk", bufs=2) as xbp,
                tc.tile_pool(name="epool", bufs=3) as ep,
                tc.tile_pool(name="rpool", bufs=1) as rp,
                tc.tile_pool(name="ostage", bufs=2) as osp,
            ):
                # ---- constants ----
                bq_sb = pers.tile([128, NDC], F32, tag="bq")
                bk_sb = pers.tile([128, NDC], F32, tag="bk")
                bv_sb = pers.tile([128, DL], F32, tag="bv")
                ones_sb = pers.tile([128, NH], F32, tag="ones")
                nc.sync.dma_start(out=bq_sb[:], in_=bq_d.ap())
                nc.sync.dma_start(out=bk_sb[:], in_=bk_d.ap())
                nc.sync.dma_start(out=bv_sb[:], in_=bv_d.ap())
                nc.any.memset(ones_sb[:], 1.0)
                if variant == "causal":
                    mt_sb = pers.tile([128, 4, 512], F32R, tag="mt")
                    for j in range(4):
                        raw = raws.tile([128, 512], F32, tag="raw", name=f"mtraw{j}")
                        nc.sync.dma_start(out=raw[:], in_=mt_d.ap()[j])
                        nc.vector.tensor_copy(mt_sb[:, j, :], raw[:])
                wo_sb = pers.tile([128, NDC, D], F32R, tag="wo")
                wo_r = wo_sb[:]
                for dl in range(NDC):
                    for hh in range(2):
                        raw = raws.tile(
                            [128, 512], F32, tag="raw", name=f"woraw{dl}_{hh}"
                        )
                        nc.sync.dma_start(
                            out=raw[:],
                            in_=wo_d.ap()[
                                dl * 128 : (dl + 1) * 128,
                                hh * 512 : (hh + 1) * 512,
                            ],
                        )
                        nc.gpsimd.tensor_copy(
                            wo_sb[:, dl, hh * 512 : (hh + 1) * 512], raw[:]
                        )

                Ksb = pers.tile([128, NDC, S], F32R, tag="Ksb")
                vts = [
                    vtp.tile([128, NH * 65], F32R, tag=f"vt{sc}", name=f"vt{sc}")
                    for sc in range(NSC)
                ]

                # ---- K projection:  Ksb[p, dc, s] = (key @ Wk_g.T).T + bk ----
                if "qk" not in skip:
                  with (
                    tc.tile_pool(name="wkp", bufs=1) as wkp,
                    tc.tile_pool(name="ppk", bufs=4, space="PSUM") as ppk,
                ):
                    wk_sb = wkp.tile([128, NDI, DL], F32R, tag="wk")
                    wk_r = wk_sb[:]
                    for di in range(NDI):
                        raw = raws.tile([128, DL], F32, tag="raw", name=f"wkraw{di}")
                        nc.sync.dma_start(
                            out=raw[:], in_=wk_d.ap()[di * 128 : (di + 1) * 128, :]
                        )
                        nc.gpsimd.tensor_copy(wk_sb[:, di, :], raw[:])
                    for sb in range(NSB):
                        kt = stream.tile([128, NDI, 512], F32R, tag="qkstream")
                        kt_r = kt[:]
                        for di in range(NDI):
                            raw = raws.tile([128, 512], F32, tag="raw", name=f"ktraw{di}")
                            nc.sync.dma_start(
                                out=raw[:],
                                in_=kT_d.ap()[
                                    di * 128 : (di + 1) * 128,
                                    sb * 512 : (sb + 1) * 512,
                                ],
                            )
                            nc.gpsimd.tensor_copy(kt[:, di, :], raw[:])
                        for dc in range(NDC):
                            ps = ppk.tile([128, 512], F32, tag="ppk")
                            for di in range(NDI):
                                nc.tensor.matmul(
                                    ps[:],
                                    wk_r[:, di, dc * 128 : (dc + 1) * 128],
                                    kt_r[:, di, :],
                                    start=(di == 0),
                                    stop=(di == NDI - 1),
                                )
                            nc.vector.tensor_scalar_add(
                                Ksb[:, dc, sb * 512 : (sb + 1) * 512],
                                ps[:],
                                bk_sb[:, dc : dc + 1],
                            )

                # ---- V projection into [V | 1] tiles ----
                if "v" not in skip:
                  with (
                    tc.tile_pool(name="wvp", bufs=1) as wvp,
                    tc.tile_pool(name="vstream", bufs=2) as vstream,
                    tc.tile_pool(name="ppv", bufs=4, space="PSUM") as ppv,
                ):
                    wv_sb = wvp.tile([128, NDI, DL], F32R, tag="wv")
                    wv_r = wv_sb[:]
                    for di in range(NDI):
                        raw = raws.tile([128, DL], F32, tag="raw", name=f"wvraw{di}")
                        nc.sync.dma_start(
                            out=raw[:], in_=wv_d.ap()[di * 128 : (di + 1) * 128, :]
                        )
                        nc.gpsimd.tensor_copy(wv_sb[:, di, :], raw[:])
                    for sc in range(NSC):
                        vtr = vstream.tile([128, NDI, 128], F32R, tag="vtr")
                        vtr_r = vtr[:]
                        for di in range(NDI):
                            raw = raws.tile([128, 128], F32, tag="raw", name=f"vraw{di}")
                            nc.sync.dma_start(
                                out=raw[:],
                                in_=vT_d.ap()[
                                    di * 128 : (di + 1) * 128,
                                    sc * 128 : (sc + 1) * 128,
                                ],
                            )
                            nc.gpsimd.tensor_copy(vtr[:, di, :], raw[:])
                        ps = ppv.tile([128, DL], F32, tag="ppv")
                        for di in range(NDI):
                            nc.tensor.matmul(
                                ps[:],
                                vtr_r[:, di, :],
                                wv_r[:, di, :],
                                start=(di == 0),
                                stop=(di == NDI - 1),
                            )
                        vt3 = vts[sc][:].rearrange("p (h c) -> p h c", h=NH)
                        nc.vector.tensor_add(
                            vt3[:, :, 0:64],
                            ps[:].rearrange("p (h c) -> p h c", h=NH),
                            bv_sb[:].rearrange("p (h c) -> p h c", h=NH),
                        )
                        nc.vector.tensor_copy(vt3[:, :, 64:65], ones_sb[:].unsqueeze(2))

                # ---- per q-block: Q proj, attention, out proj ----
                with (
                    tc.tile_pool(name="wqp", bufs=1) as wqp,
                    tc.tile_pool(name="mq", bufs=1) as mqp,
                    tc.tile_pool(name="pss", bufs=3, space="PSUM") as pss,
                    tc.tile_pool(name="pso", bufs=1, space="PSUM") as pso,
                ):
                    wq_sb = wqp.tile([128, NDI, DL], F32R, tag="wq")
                    wq_r = wq_sb[:]
                    for di in range(NDI):
                        raw = raws.tile([128, DL], F32, tag="raw", name=f"wqraw{di}")
                        nc.sync.dma_start(
                            out=raw[:], in_=wq_d.ap()[di * 128 : (di + 1) * 128, :]
                        )
                        nc.gpsimd.tensor_copy(wq_sb[:, di, :], raw[:])
                    env = dict(
                        skip=skip,
                        nc=nc, variant=variant, kept_kcs=kept_kcs,
                        qT_d=qT_d, mT_d=(mT_d if variant == "general" else None),
                        outT_d=outT_d,
                        stream=stream, raws=raws, qbp=qbp, xbp=xbp, ep=ep,
                        rp=rp, osp=osp, mqp=mqp,
                        pss=pss, pso=pso,
                        wq_r=wq_r, wo_r=wo_r, Ksb=Ksb, vts=vts,
                        bq_sb=bq_sb,
                        mt_sb=(mt_sb if variant == "causal" else None),
                    )
                    for qb in range(NSB):
                        _emit_qblock(env, locals(), qb)
    nc.compile()
    return nc


# ---------------------------------------------------------------------------
# host side
# ---------------------------------------------------------------------------

_NC_CACHE = {}


def _get_program(variant, reps=1):
    key = (variant, reps)
    if key not in _NC_CACHE:
        _NC_CACHE[key] = build_program(variant, reps)
    return _NC_CACHE[key]


def detect_variant(mask):
    m = np.asarray(mask)
    if (m != 0).all():
        return "ones"
    tril = np.tril(np.ones((S, S), np.int8))
    for b in range(m.shape[0]):
        mb = (m[b] != 0).astype(np.int8)
        if not np.array_equal(mb, tril):
            return "general"
    return "causal"


def make_causal_mask_tiles():
    j = np.arange(4)[:, None, None]
    k = np.arange(128)[None, :, None]
    q = np.arange(512)[None, None, :]
    # multiplicative: 1 keep, 0 drop (applied to exp'd scores)
    return (q >= k + 128 * j).astype(np.float32)


def build_in_maps(query, key, value, mask, Wq, bq, Wk, bk, Wv, bv, Wo, bo, variant):
    query = np.asarray(query, np.float32)
    key = np.asarray(key, np.float32)
    value = np.asarray(value, np.float32)
    Wq, Wk, Wv, Wo = (np.asarray(w, np.float32) for w in (Wq, Wk, Wv, Wo))
    bq, bk, bv = (np.asarray(x, np.float32) for x in (bq, bk, bv))

    if variant == "causal":
        mtiles = make_causal_mask_tiles()

    in_maps = []
    for c in range(8):
        b, g = c // 2, c % 2
        gs = slice(g * DL, (g + 1) * DL)
        m = {
            "qT": np.ascontiguousarray(query[b].T),
            "kT": np.ascontiguousarray(key[b].T),
            "vT": np.ascontiguousarray(value[b].T),
            "wq": np.ascontiguousarray(Wq[gs].T),
            "wk": np.ascontiguousarray(Wk[gs].T),
            "wv": np.ascontiguousarray(Wv[gs].T),
            "wo": np.ascontiguousarray(Wo[:, gs].T),
            "bq": np.ascontiguousarray(bq[gs].reshape(NDC, 128).T),
            "bk": np.ascontiguousarray(bk[gs].reshape(NDC, 128).T),
            "bv": np.ascontiguousarray(np.broadcast_to(bv[gs], (128, DL))),
        }
        if variant == "causal":
            m["maskt"] = mtiles
        elif variant == "general":
            m["maskT"] = np.ascontiguousarray(
                np.where(np.asarray(mask[b]) != 0, 0.0, NEG).astype(np.float32).T
            )
        in_maps.append(m)
    return in_maps


def assemble_output(results, bo):
    bo = np.asarray(bo, np.float32)
    out = np.empty((B, S, D), np.float32)
    for b in range(B):
        acc = results[2 * b]["outT"] + results[2 * b + 1]["outT"]
        out[b] = acc.T + bo
    return out


def kernel(query, key, value, mask, Wq, bq, Wk, bk, Wv, bv, Wo, bo):
    variant = detect_variant(np.asarray(mask))
    in_maps = build_in_maps(
        query, key, value, mask, Wq, bq, Wk, bk, Wv, bv, Wo, bo, variant
    )
    nc = _get_program(variant)
    res = run_bass_kernel_spmd(nc, in_maps, core_ids=list(range(8)))
    return assemble_output(res.results, bo)
